# revision 44
# baseline (speedup 1.0000x reference)
"""Segment-mean (sentence pooling) Bass/Tile kernel for Trainium2.

Problem: last_hidden_state [16, 4096, 1024] f32, sentence_mask [16, 4096] int,
num_sents=32. For each (batch, sentence id): mean of hidden states at seq
positions whose mask equals the id. Returns (embeddings [16, 32, 1024] f32,
unique_sents [32] int).

Strategy: data-parallel over batch across 8 NeuronCores (2 batches/core).
Each core streams seq-chunks of hidden states via DMA and computes the
segment sum as a matmul against a host-built one-hot mask chunk [128, 32]
(stationary lhsT), accumulating over chunks in PSUM.
Memory-bound: 32 MiB/core of hidden states.

Modes:
- "split16" (default): exact-fp32 result at fp16 PE cost with zero on-chip
  pre-processing. The HOST splits each f32 element into hi = fp16(x) and
  lo = fp16(x - hi) (4 bytes total - same DMA traffic as f32); the device
  streams the two fp16 planes of both batches in shared 1 MiB DMAs straight
  into matmuls. Batch b's matmuls write PSUM partitions [b*32,(b+1)*32)
  (PE col-group b) so batch pairs execute concurrently in the array. One-hot
  weights are 0/1 in fp16 (exact); 1/count scaling is applied on the
  PSUM->SBUF copy. fp16 splitting carries 22 mantissa bits; error is at the
  fp32-accumulation-envelope level (~3e-7 relative).
- "f32": plain fp32 matmul (exact but 4 PE cycles/row; PE-bound).
- "f32r": fp32r matmul (fast but ~2e-4 relative error).
"""

import numpy as np

BATCH, SEQ, HID, NS = 16, 4096, 1024, 32
P = 128
NCORES = 8
BPC = BATCH // NCORES  # batches per core
CHUNKS = SEQ // P  # seq chunks of 128
CPD = 2  # chunks per DMA (1 MiB DMAs)

MODE = "split16"
HBUFS = 10  # hidden-tile buffer depth (x 1 MiB)

_nc_cache = {}


def _make_tile_context(tile_mod, nc):
    """TileContext whose exit skips the on-device semaphore clears + second
    all-engine barrier: the NEFF epilogue already zeroes every semaphore, so
    the Tile-level RANGE_CLEAR round trip (~2us) is redundant."""
    from concourse.vector_clock import ScopedClock

    class FastTileContext(tile_mod.TileContext):
        def _drain_and_barrier(self, tick_clock, wait_clock):
            drain_inst = self.nc.sync.drain()
            wait_clock.add_sem_waits(
                drain_inst.ins, ScopedClock({None: tick_clock.global_clock})
            )
            self.nc.all_engine_barrier()
            popped = self.nc._tile_sem_poison_stack.pop()
            assert popped is self._sem_poison
            sems = list(self.sems.allocated().values())
            sem_nums = [s.num if hasattr(s, "num") else s for s in sems]
            self.nc._state.prepend_free_semaphores(sem_nums)
            for poison_set in self.nc._tile_sem_poison_stack:
                poison_set.update(sem_nums)

    return FastTileContext(nc)


def _build_nc(mode):
    import concourse.mybir as mybir
    import concourse.tile as tile
    from concourse import bacc

    nc = bacc.Bacc(
        "TRN2", target_bir_lowering=False, debug=False, enable_asserts=False
    )
    f32 = mybir.dt.float32
    f16 = mybir.dt.float16

    o = nc.dram_tensor("o", [BPC, NS, HID], f32, kind="ExternalOutput")
    w_dt = f16 if mode == "split16" else f32
    w = nc.dram_tensor("w", [BPC, P, CHUNKS, NS], w_dt, kind="ExternalInput")
    if mode == "split16":
        # host-pre-split fp16 hi/lo planes: [c, p, b, part(hi/lo), d]
        # (hi+lo = 4 B/elem, same DMA bytes as the f32 input)
        x16 = nc.dram_tensor(
            "x16", [CHUNKS, P, BPC, 2, HID], f16, kind="ExternalInput"
        )
        invc = nc.dram_tensor("invc", [BPC, NS, 1], f32, kind="ExternalInput")
    else:
        h = nc.dram_tensor("h", [BPC, SEQ, HID], f32, kind="ExternalInput")
        # [b, g, p, q, d]: DMA tile g covers chunks (2g, 2g+1)
        h5 = h.ap().rearrange("b (g q p) d -> b g p q d", q=CPD, p=P)

    sb_dt = mybir.dt.float32r if mode == "f32r" else f32

    with _make_tile_context(tile, nc) as tc:
        with (
            tc.tile_pool(name="wp", bufs=1) as wp,
            tc.tile_pool(name="hp", bufs=HBUFS) as hp,
            tc.tile_pool(name="sp", bufs=8) as sp,
            tc.tile_pool(name="op", bufs=2) as op,
            tc.tile_pool(name="pp", bufs=2 * BPC, space="PSUM") as pp,
        ):
            if mode == "split16":
                # warm the ACT Copy table off the critical path: the first
                # ACTIVATE pays a ~1.5us ACT_TABLE_LOAD
                warm = wp.tile([P, 8], mybir.dt.float32, tag="warm", name="warm")
                nc.vector.memset(warm[:], 0.0)
                warm16 = wp.tile([P, 8], f16, tag="warm16", name="warm16")
                nc.scalar.copy(warm16[:], warm[:])
            wts = []
            for b in range(BPC):
                wt = wp.tile([P, CHUNKS, NS], w_dt, tag=f"w{b}", name=f"w{b}")
                if mode == "f32r":
                    nc.gpsimd.dma_start(wt[:], w.ap()[b])  # casts f32 -> f32r
                else:
                    # scalar HWDGE ring: keep the SP ring a pure h-stream
                    nc.scalar.dma_start(wt[:], w.ap()[b])
                wts.append(wt)

            if mode == "split16":
                # host-pre-split fp16 stream: no ACT/DVE work in the stream.
                # Each 1 MiB DMA carries chunk c of both batches (hi+lo);
                # batch b's matmuls target PSUM partitions [b*NS,(b+1)*NS)
                # = PE col-group b, so (b0,b1) pairs run concurrently.
                ict = wp.tile([BPC * NS, 1], f32, tag="ic", name="ic")
                nc.scalar.dma_start(
                    ict[:], invc.ap().rearrange("b s one -> (b s) one")
                )
                ps = [
                    pp.tile([BPC * NS, 512], f32, tag=f"ps{n}", name=f"ps{n}")
                    for n in range(2)
                ]
                for c in range(CHUNKS):
                    xt = hp.tile([P, BPC, 2, HID], f16, tag="h", name=f"x{c}")
                    nc.sync.dma_start(xt[:], x16.ap()[c])
                    for part in range(2):
                        for n in range(2):
                            for b in range(BPC):
                                nc.tensor.matmul(
                                    ps[n][b * NS : (b + 1) * NS, :],
                                    wts[b][:, c, :],
                                    xt[:, b, part, n * 512 : (n + 1) * 512],
                                    start=(c == 0 and part == 0),
                                    stop=(c == CHUNKS - 1 and part == 1),
                                )
                ot = op.tile([BPC * NS, HID], f32, tag="o", name="ot")
                for n in range(2):
                    nc.vector.tensor_scalar_mul(
                        ot[:, n * 512 : (n + 1) * 512], ps[n][:], ict[:]
                    )
                nc.scalar.dma_start(o.ap().rearrange("b s d -> (b s) d"), ot[:])
            else:
                for b in range(BPC):
                    ps = [
                        pp.tile([NS, 512], f32, tag=f"ps{n}", name=f"ps{b}_{n}")
                        for n in range(2)
                    ]
                    for g in range(CHUNKS // CPD):
                        ht = hp.tile([P, CPD, HID], sb_dt, tag="h", name=f"h{b}_{g}")
                        if mode == "f32r":
                            nc.gpsimd.dma_start(ht[:], h5[b, g])
                        else:
                            nc.sync.dma_start(ht[:], h5[b, g])
                        for q in range(CPD):
                            c = g * CPD + q
                            for n in range(2):
                                nc.tensor.matmul(
                                    ps[n][:],
                                    wts[b][:, c, :],
                                    ht[:, q, n * 512 : (n + 1) * 512],
                                    start=(c == 0),
                                    stop=(c == CHUNKS - 1),
                                )
                    ot = op.tile([NS, HID], f32, tag="o", name=f"o{b}")
                    for n in range(2):
                        nc.vector.tensor_copy(ot[:, n * 512 : (n + 1) * 512], ps[n][:])
                    nc.scalar.dma_start(o.ap()[b], ot[:])
    nc.compile()
    return nc


def _get_nc(mode=None):
    mode = mode or MODE
    if mode not in _nc_cache:
        _nc_cache[mode] = _build_nc(mode)
    return _nc_cache[mode]


def _prepare(last_hidden_state, sentence_mask, num_sents, mode=None):
    """Host prep: shard on batch, build the one-hot mask tensor."""
    mode = mode or MODE
    lhs = np.ascontiguousarray(np.asarray(last_hidden_state, dtype=np.float32))
    mask = np.asarray(sentence_mask)
    ns = int(num_sents)
    assert lhs.shape == (BATCH, SEQ, HID) and ns == NS, (lhs.shape, ns)

    ids = mask.astype(np.int64)
    onehot = ids[:, :, None] == np.arange(ns, dtype=np.int64)[None, None, :]
    counts = onehot.sum(axis=1)  # [B, NS]
    inv = (1.0 / np.maximum(counts, 1)).astype(np.float32)
    if mode == "split16":
        w = onehot.astype(np.float16)  # 0/1, exact
    else:
        w = onehot.astype(np.float32) * inv[:, None, :]
    # [B, S, NS] -> [B, P, CHUNKS, NS] with S = c*P + p
    w = np.ascontiguousarray(w.reshape(BATCH, CHUNKS, P, ns).transpose(0, 2, 1, 3))

    if mode == "split16":
        # host-side exact fp16 hi/lo split (hi+lo = 4 B/elem, same bytes
        # as f32): device streams two fp16 planes straight into matmuls
        hi = lhs.astype(np.float16)
        lo = (lhs - hi.astype(np.float32)).astype(np.float16)
        # [B, S, D] -> [B, C, P, D] -> stack part -> [B, C, P, 2, D]
        x = np.stack(
            [
                hi.reshape(BATCH, CHUNKS, P, HID),
                lo.reshape(BATCH, CHUNKS, P, HID),
            ],
            axis=3,
        )  # [B, C, P, 2, D]

    in_maps = []
    for i in range(NCORES):
        m = {"w": w[i * BPC : (i + 1) * BPC]}
        if mode == "split16":
            # [b, C, P, 2, D] -> [C, P, b, 2, D]
            m["x16"] = np.ascontiguousarray(
                x[i * BPC : (i + 1) * BPC].transpose(1, 2, 0, 3, 4)
            )
            m["invc"] = np.ascontiguousarray(
                inv[i * BPC : (i + 1) * BPC, :, None]
            )
        else:
            m["h"] = lhs[i * BPC : (i + 1) * BPC]
        in_maps.append(m)
    return in_maps, mask.dtype


def _execute(in_maps, trace=False, mode=None, **kwargs):
    from concourse.bass_utils import run_bass_kernel_spmd

    return run_bass_kernel_spmd(
        _get_nc(mode),
        in_maps,
        core_ids=list(range(NCORES)),
        trace=trace,
        **kwargs,
    )


def _gather(results):
    return np.concatenate([r["o"] for r in results], axis=0)


def kernel(last_hidden_state, sentence_mask, num_sents):
    in_maps, mask_dtype = _prepare(last_hidden_state, sentence_mask, num_sents)
    res = _execute(in_maps)
    emb = _gather(res.results)
    unique_sents = np.arange(int(num_sents), dtype=mask_dtype)
    return emb, unique_sents


# revision 47
# speedup vs baseline: 1.0922x; 1.0922x over previous
"""Segment-mean (sentence pooling) Bass/Tile kernel for Trainium2.

Problem: last_hidden_state [16, 4096, 1024] f32, sentence_mask [16, 4096] int,
num_sents=32. For each (batch, sentence id): mean of hidden states at seq
positions whose mask equals the id. Returns (embeddings [16, 32, 1024] f32,
unique_sents [32] int).

Strategy: data-parallel over batch across 8 NeuronCores (2 batches/core).
Each core streams seq-chunks of hidden states via DMA and computes the
segment sum as a matmul against a host-built one-hot mask chunk [128, 32]
(stationary lhsT), accumulating over chunks in PSUM.
Memory-bound: 32 MiB/core of hidden states.

Modes:
- "split16" (default): exact-fp32 result at fp16 PE cost with zero on-chip
  pre-processing. The HOST splits each f32 element into hi = fp16(x) and
  lo = fp16(x - hi) (4 bytes total - same DMA traffic as f32); the device
  streams the two fp16 planes of both batches in shared 1 MiB DMAs straight
  into matmuls. Batch b's matmuls write PSUM partitions [b*32,(b+1)*32)
  (PE col-group b) so batch pairs execute concurrently in the array. One-hot
  weights are 0/1 in fp16 (exact); 1/count scaling is applied on the
  PSUM->SBUF copy. fp16 splitting carries 22 mantissa bits; error is at the
  fp32-accumulation-envelope level (~3e-7 relative).
- "f32": plain fp32 matmul (exact but 4 PE cycles/row; PE-bound).
- "f32r": fp32r matmul (fast but ~2e-4 relative error).
"""

import numpy as np

BATCH, SEQ, HID, NS = 16, 4096, 1024, 32
P = 128
NCORES = 8
BPC = BATCH // NCORES  # batches per core
CHUNKS = SEQ // P  # seq chunks of 128
CPD = 2  # chunks per DMA (1 MiB DMAs)

MODE = "split16"
HBUFS = 10  # hidden-tile buffer depth (x 1 MiB)

_nc_cache = {}


def _make_tile_context(tile_mod, nc):
    """TileContext whose exit skips the on-device semaphore clears + second
    all-engine barrier: the NEFF epilogue already zeroes every semaphore, so
    the Tile-level RANGE_CLEAR round trip (~2us) is redundant."""
    from concourse.vector_clock import ScopedClock

    class FastTileContext(tile_mod.TileContext):
        def _drain_and_barrier(self, tick_clock, wait_clock):
            drain_inst = self.nc.sync.drain()
            wait_clock.add_sem_waits(
                drain_inst.ins, ScopedClock({None: tick_clock.global_clock})
            )
            self.nc.all_engine_barrier()
            popped = self.nc._tile_sem_poison_stack.pop()
            assert popped is self._sem_poison
            sems = list(self.sems.allocated().values())
            sem_nums = [s.num if hasattr(s, "num") else s for s in sems]
            self.nc._state.prepend_free_semaphores(sem_nums)
            for poison_set in self.nc._tile_sem_poison_stack:
                poison_set.update(sem_nums)

    return FastTileContext(nc)


def _build_nc(mode):
    import concourse.mybir as mybir
    import concourse.tile as tile
    from concourse import bacc

    nc = bacc.Bacc(
        "TRN2", target_bir_lowering=False, debug=False, enable_asserts=False
    )
    f32 = mybir.dt.float32
    f16 = mybir.dt.float16

    o = nc.dram_tensor("o", [BPC, NS, HID], f32, kind="ExternalOutput")
    w_dt = f16 if mode == "split16" else f32
    w = nc.dram_tensor("w", [BPC, P, CHUNKS, NS], w_dt, kind="ExternalInput")
    if mode == "split16":
        # host-pre-split fp16 hi/lo planes: [c, p, b, part(hi/lo), d]
        # (hi+lo = 4 B/elem, same DMA bytes as the f32 input)
        x16 = nc.dram_tensor(
            "x16", [CHUNKS, P, BPC, 2, HID], f16, kind="ExternalInput"
        )
        invc = nc.dram_tensor("invc", [BPC, NS, 1], f32, kind="ExternalInput")
    else:
        h = nc.dram_tensor("h", [BPC, SEQ, HID], f32, kind="ExternalInput")
        # [b, g, p, q, d]: DMA tile g covers chunks (2g, 2g+1)
        h5 = h.ap().rearrange("b (g q p) d -> b g p q d", q=CPD, p=P)

    sb_dt = mybir.dt.float32r if mode == "f32r" else f32

    with _make_tile_context(tile, nc) as tc:
        with (
            tc.tile_pool(name="wp", bufs=1) as wp,
            tc.tile_pool(name="hp", bufs=HBUFS) as hp,
            tc.tile_pool(name="sp", bufs=8) as sp,
            tc.tile_pool(name="op", bufs=2) as op,
            tc.tile_pool(name="pp", bufs=2 * BPC, space="PSUM") as pp,
        ):
            if mode == "split16":
                # warm the ACT Copy table off the critical path: the first
                # ACTIVATE pays a ~1.5us ACT_TABLE_LOAD
                warm = wp.tile([P, 8], mybir.dt.float32, tag="warm", name="warm")
                nc.vector.memset(warm[:], 0.0)
                warm16 = wp.tile([P, 8], f16, tag="warm16", name="warm16")
                nc.scalar.copy(warm16[:], warm[:])
            wts = []
            for b in range(BPC):
                wt = wp.tile([P, CHUNKS, NS], w_dt, tag=f"w{b}", name=f"w{b}")
                if mode == "f32r":
                    nc.gpsimd.dma_start(wt[:], w.ap()[b])  # casts f32 -> f32r
                else:
                    # scalar ring: frees the sync ring so the x stream starts
                    # ~1.3us earlier; the first matmuls may start late but PE
                    # catches up at ~0.6us/chunk (stream is DMA-paced)
                    nc.scalar.dma_start(wt[:], w.ap()[b])
                wts.append(wt)

            if mode == "split16":
                # host-pre-split fp16 stream: no ACT/DVE work in the stream.
                # Each 1 MiB DMA carries chunk c of both batches (hi+lo);
                # batch b's matmuls target PSUM partitions [b*NS,(b+1)*NS)
                # = PE col-group b, so (b0,b1) pairs run concurrently.
                ict = wp.tile([BPC * NS, 1], f32, tag="ic", name="ic")
                nc.scalar.dma_start(
                    ict[:], invc.ap().rearrange("b s one -> (b s) one")
                )
                ps = [
                    pp.tile([BPC * NS, 512], f32, tag=f"ps{n}", name=f"ps{n}")
                    for n in range(2)
                ]
                ot = op.tile([BPC * NS, HID], f32, tag="o", name="ot")
                o_flat = o.ap().rearrange("b s d -> (b s) d")
                for c in range(CHUNKS - 1):
                    xt = hp.tile([P, BPC, 2, HID], f16, tag="h", name=f"x{c}")
                    nc.sync.dma_start(xt[:], x16.ap()[c])
                    for part in range(2):
                        for n in range(2):
                            for b in range(BPC):
                                nc.tensor.matmul(
                                    ps[n][b * NS : (b + 1) * NS, :],
                                    wts[b][:, c, :],
                                    xt[:, b, part, n * 512 : (n + 1) * 512],
                                    start=(c == 0 and part == 0),
                                    stop=False,
                                )
                # last chunk: two 512 KB part-DMAs so the hi-plane matmuls run
                # while the lo plane is still in flight; each bank's stop
                # matmul is chased by its scale + output half-DMA
                cl = CHUNKS - 1
                xl = []
                for part in range(2):
                    xp = op.tile(
                        [P, BPC, 1, HID], f16, tag=f"hl{part}", name=f"xl{part}"
                    )
                    nc.sync.dma_start(xp[:], x16.ap()[cl][:, :, part : part + 1, :])
                    xl.append(xp)
                for part in range(2):
                    for n in range(2):
                        for b in range(BPC):
                            nc.tensor.matmul(
                                ps[n][b * NS : (b + 1) * NS, :],
                                wts[b][:, cl, :],
                                xl[part][:, b, 0, n * 512 : (n + 1) * 512],
                                start=False,
                                stop=(part == 1),
                            )
                        if part == 1:
                            nsl = slice(n * 512, (n + 1) * 512)
                            nc.vector.tensor_scalar_mul(ot[:, nsl], ps[n][:], ict[:])
                            nc.scalar.dma_start(o_flat[:, nsl], ot[:, nsl])
            else:
                for b in range(BPC):
                    ps = [
                        pp.tile([NS, 512], f32, tag=f"ps{n}", name=f"ps{b}_{n}")
                        for n in range(2)
                    ]
                    for g in range(CHUNKS // CPD):
                        ht = hp.tile([P, CPD, HID], sb_dt, tag="h", name=f"h{b}_{g}")
                        if mode == "f32r":
                            nc.gpsimd.dma_start(ht[:], h5[b, g])
                        else:
                            nc.sync.dma_start(ht[:], h5[b, g])
                        for q in range(CPD):
                            c = g * CPD + q
                            for n in range(2):
                                nc.tensor.matmul(
                                    ps[n][:],
                                    wts[b][:, c, :],
                                    ht[:, q, n * 512 : (n + 1) * 512],
                                    start=(c == 0),
                                    stop=(c == CHUNKS - 1),
                                )
                    ot = op.tile([NS, HID], f32, tag="o", name=f"o{b}")
                    for n in range(2):
                        nc.vector.tensor_copy(ot[:, n * 512 : (n + 1) * 512], ps[n][:])
                    nc.scalar.dma_start(o.ap()[b], ot[:])
    nc.compile()
    return nc


def _get_nc(mode=None):
    mode = mode or MODE
    if mode not in _nc_cache:
        _nc_cache[mode] = _build_nc(mode)
    return _nc_cache[mode]


def _prepare(last_hidden_state, sentence_mask, num_sents, mode=None):
    """Host prep: shard on batch, build the one-hot mask tensor."""
    mode = mode or MODE
    lhs = np.ascontiguousarray(np.asarray(last_hidden_state, dtype=np.float32))
    mask = np.asarray(sentence_mask)
    ns = int(num_sents)
    assert lhs.shape == (BATCH, SEQ, HID) and ns == NS, (lhs.shape, ns)

    ids = mask.astype(np.int64)
    onehot = ids[:, :, None] == np.arange(ns, dtype=np.int64)[None, None, :]
    counts = onehot.sum(axis=1)  # [B, NS]
    inv = (1.0 / np.maximum(counts, 1)).astype(np.float32)
    if mode == "split16":
        w = onehot.astype(np.float16)  # 0/1, exact
    else:
        w = onehot.astype(np.float32) * inv[:, None, :]
    # [B, S, NS] -> [B, P, CHUNKS, NS] with S = c*P + p
    w = np.ascontiguousarray(w.reshape(BATCH, CHUNKS, P, ns).transpose(0, 2, 1, 3))

    if mode == "split16":
        # host-side exact fp16 hi/lo split (hi+lo = 4 B/elem, same bytes
        # as f32): device streams two fp16 planes straight into matmuls
        hi = lhs.astype(np.float16)
        lo = (lhs - hi.astype(np.float32)).astype(np.float16)
        # [B, S, D] -> [B, C, P, D] -> stack part -> [B, C, P, 2, D]
        x = np.stack(
            [
                hi.reshape(BATCH, CHUNKS, P, HID),
                lo.reshape(BATCH, CHUNKS, P, HID),
            ],
            axis=3,
        )  # [B, C, P, 2, D]

    in_maps = []
    for i in range(NCORES):
        m = {"w": w[i * BPC : (i + 1) * BPC]}
        if mode == "split16":
            # [b, C, P, 2, D] -> [C, P, b, 2, D]
            m["x16"] = np.ascontiguousarray(
                x[i * BPC : (i + 1) * BPC].transpose(1, 2, 0, 3, 4)
            )
            m["invc"] = np.ascontiguousarray(
                inv[i * BPC : (i + 1) * BPC, :, None]
            )
        else:
            m["h"] = lhs[i * BPC : (i + 1) * BPC]
        in_maps.append(m)
    return in_maps, mask.dtype


def _execute(in_maps, trace=False, mode=None, **kwargs):
    from concourse.bass_utils import run_bass_kernel_spmd

    return run_bass_kernel_spmd(
        _get_nc(mode),
        in_maps,
        core_ids=list(range(NCORES)),
        trace=trace,
        **kwargs,
    )


def _gather(results):
    return np.concatenate([r["o"] for r in results], axis=0)


def kernel(last_hidden_state, sentence_mask, num_sents):
    in_maps, mask_dtype = _prepare(last_hidden_state, sentence_mask, num_sents)
    res = _execute(in_maps)
    emb = _gather(res.results)
    unique_sents = np.arange(int(num_sents), dtype=mask_dtype)
    return emb, unique_sents
